# revision 38
# baseline (speedup 1.0000x reference)
"""Conditional (class-routed) 3x3 SAME conv, data-parallel over batch on 8 TRN2 cores.

Strategy (v3 - fp8 DoubleRow residual scheme, per-row windows):
  - Zero-padded x [CIN, 66, 66] stored as flat [CIN, 4356] planes; each
    output row's tap window x[r+kh, kw:kw+64] is a contiguous 64-elem slice
    of the flat plane, so matmuls use clean 3-dim [K, 2, 64] access patterns
    and compute exactly the 4096 true output positions (row chunks of
    2/8/../4/2 rows accumulate per-row 64-wide matmuls into one PSUM bank).
    Taps that read only zero padding (kh=0 at row 0, kh=2 at row 63) are
    skipped.
  - fp8e4 (e4m3) matmuls in MatmulPerfMode.DoubleRow: lhsT [K,2,M] x
    rhs [K,2,N] -> out[M,N] contracts a PAIR of products per output at half
    the per-row cost of fp16 -> 4x cheaper per product-term.
  - Residual precision scheme (kernel pre-scaled by 64, split k = k_hi+k_lo
    and x = x_hi+x_lo in e4m3):
      MM_A(tap t in 0..8): slots (x_hi*k_hi[t], x_lo*k_hi[t]) - exact-x
      MM_C(p in 0..2):     slots (x_hi*k_lo[t=p], x_hi*k_lo[t=p+3]) -
        k_lo corrections for the kh=0 and kh=1 tap rows, packed 2 per matmul
        using an extra "x_hi shifted by 66" SBUF plane (window offsets of the
        paired taps differ by exactly one padded row).
    Dropped terms: x*k_lo on the kh=2 tap row (dominant error, ~1.6e-2 rel)
    and x_lo*k_lo (~7e-4).  12 DoubleRow matmul kinds per row vs 9 fp16
    matmuls: ~0.75x the fp16 PE-roofline time.
  - PSUM eviction applies out = psum/64 + bias, alternating DVE/ACT engines.
  - Host: gather per-sample expert kernels, build hi/lo e4m3 splits + the
    shifted plane, and concatenate shards.
"""

import numpy as np

_B, _H, _W, _CIN = 32, 64, 64, 128
_F, _KH, _KW = 256, 3, 3
_NCORES = 8
_BPC = _B // _NCORES          # 4 samples per core
_HP, _WP = _H + 2, _W + 2     # 66, 66 (zero-padded)
_FLAT = _HP * _WP             # 4356 flat padded positions
_FH = 128                     # output-channel half (PSUM partition dim)
_NFH = _F // _FH              # 2
_NTAP = _KH * _KW             # 9
_KSCALE = 64.0                # kernel pre-scale (keeps e4m3 out of subnormals)

_OFFS = [kh * _WP + kw for kh in range(_KH) for kw in range(_KW)]
_NPAIR = 3                    # MM_C count: pair p corrects taps (p, p+3)
_NMM = _NTAP + _NPAIR         # 12 matmul kinds per chunk (x 1 per row)
# row-granular chunks (per-row N=64 windows skip the 2 pad columns per row):
# tiny first chunk -> early PE start, tiny last chunk -> short drain tail
_ROWCHUNKS = ((0, 2), (2, 10), (10, 18), (18, 26), (26, 34), (34, 42),
              (42, 50), (50, 58), (58, 63), (63, 64))
_XSPLITS = (0, 512, 1060, 1810, 2560, 3560, _FLAT)  # x DMA piece boundaries

_nc = None
_E4M3 = None


def _e4m3():
    global _E4M3
    if _E4M3 is None:
        import concourse.mybir as mybir
        _E4M3 = np.dtype(mybir.dt.np(mybir.dt.float8e4))
    return _E4M3


def _build_nc():
    import concourse.bacc as bacc
    import concourse.mybir as mybir
    import concourse.tile as tile
    from concourse.tile_rust import add_dep_helper

    f32 = mybir.dt.float32
    f16 = mybir.dt.float16
    f8 = mybir.dt.float8e4
    DR = mybir.MatmulPerfMode.DoubleRow
    ident = mybir.ActivationFunctionType.Identity
    mult, add = mybir.AluOpType.mult, mybir.AluOpType.add

    nc = bacc.Bacc("TRN2", target_bir_lowering=False, debug=False)
    # x planes per sample: [CIN, 3(hi, lo, hi<<66), 4356] e4m3
    xT = nc.dram_tensor("xT", (_BPC, _CIN, 3, _FLAT), f8, kind="ExternalInput")
    # kernel tiles per (s, fh): [CIN, 12, 2, FH] e4m3 (see host prep)
    kT = nc.dram_tensor("kT", (_BPC, _NFH, _CIN, _NMM, 2, _FH), f8,
                        kind="ExternalInput")
    bT = nc.dram_tensor("bT", (_FH, _BPC * _NFH), f32, kind="ExternalInput")
    yT = nc.dram_tensor("yT", (_BPC, _NFH, _FH, _H * _W), f16,
                        kind="ExternalOutput")

    with tile.TileContext(nc) as tc:
        with (
            tc.tile_pool(name="xp", bufs=2) as xp,
            tc.tile_pool(name="kp", bufs=4) as kp,
            tc.tile_pool(name="bp", bufs=1) as bp,
            tc.tile_pool(name="op", bufs=4) as op,
            tc.tile_pool(name="osp", bufs=2) as osp,
            tc.tile_pool(name="ps", bufs=7, space="PSUM") as ps,
            tc.tile_pool(name="pss", bufs=1, space="PSUM") as pss,
        ):
            b_sb = None
            gate_prev = None
            for s in range(_BPC):
                dmas = []
                k_sb = []
                x_sb = xp.tile([_CIN, 3, _FLAT], f8, name=f"x{s}", tag="x")

                def load_k(fh, lo, hi, eng=None, s=s, k_sb=k_sb, dmas=dmas):
                    if lo == 0:
                        t = kp.tile([_CIN, _NMM, 2, _FH], f8,
                                    name=f"k{s}f{fh}", tag="k")
                        k_sb.append(t)
                    else:
                        t = k_sb[fh]
                    dmas.append((eng or nc.sync).dma_start(
                        t[:, lo:hi], kT[s, fh, :, lo:hi]))

                def load_x(a, bnd, eng=None, s=s, x_sb=x_sb, dmas=dmas):
                    dmas.append((eng or nc.sync).dma_start(
                        x_sb[:, :, a:bnd], xT[s, :, :, a:bnd]))

                if s == 0:
                    # SP queue only (transfers serialize on the DMA device
                    # anyway; the ACT queue is blocked early by the
                    # LoadActFuncSet).  Order = need order; everything past
                    # xB/bias is gated behind the first matmul.
                    load_k(0, 0, 2)
                    load_x(0, _XSPLITS[1])
                    load_k(0, 2, 7)
                    load_k(0, 7, _NMM)
                    load_x(_XSPLITS[1], _XSPLITS[2])
                    n_crit = len(dmas)
                    load_x(_XSPLITS[2], _XSPLITS[3])
                    b_sb = bp.tile([_FH, _BPC * _NFH], f32)
                    dmas.append(nc.sync.dma_start(b_sb[:], bT[:]))
                    for piece in range(3, len(_XSPLITS) - 1):
                        load_x(_XSPLITS[piece], _XSPLITS[piece + 1])
                    load_k(1, 0, _NMM)
                else:
                    # prefetch: k tiles via the ACT queue to offload SP
                    load_k(0, 0, _NMM, nc.scalar)
                    load_x(0, 2178)
                    load_x(2178, _FLAT)
                    load_k(1, 0, _NMM, nc.scalar)

                if gate_prev is not None:
                    # prefetch of sample s must not compete for HBM bandwidth
                    # with sample s-1's (still critical) input transfers
                    for d in dmas:
                        add_dep_helper(d.ins, gate_prev,
                                       reason="prefetch gated on prev sample")
                else:
                    # sample 0: keep late pieces off the wire until compute
                    # has started so the critical prefix gets full bandwidth
                    late = dmas[n_crit:]

                gate_this = None
                for fh in range(_NFH):
                    col = s * _NFH + fh
                    bias_ap = b_sb[:, col:col + 1]
                    o_pair = None
                    nbig = 0
                    nlast = len(_ROWCHUNKS) - 1
                    for ci, (r0, r1) in enumerate(_ROWCHUNKS):
                        nr = r1 - r0
                        n = nr * _W
                        base = r0 * _W
                        tiny = nr <= 2
                        pool = pss if tiny else ps
                        psum = pool.tile([_FH, n], f32,
                                         name=f"ps_s{s}f{fh}c{ci}",
                                         tag="pss" if tiny else "psum")
                        started = False
                        for j in range(_NTAP):
                            for r in range(r0, r1):
                                # taps reading only zero padding contribute
                                # nothing: kh=0 at output row 0, kh=2 at 63
                                if ((r == 0 and j < _KW)
                                        or (r == _H - 1 and j >= 2 * _KW)):
                                    continue
                                o = r * _WP + _OFFS[j]
                                rr = (r - r0) * _W
                                mm = nc.tensor.matmul(
                                    psum[:, rr:rr + _W], k_sb[fh][:, j],
                                    x_sb[:, 0:2, o:o + _W],
                                    start=not started, stop=False,
                                    perf_mode=DR,
                                )
                                if not started:
                                    if (gate_prev is None and s == 0
                                            and fh == 0 and ci == 0):
                                        for d in late:
                                            add_dep_helper(
                                                d.ins, mm.ins,
                                                reason="s0 late after 1st MM")
                                    if fh == 0 and ci == 3:
                                        gate_this = mm.ins
                                started = True
                        for p in range(_NPAIR):
                            for r in range(r0, r1):
                                # slots (x_hi[o], x_hi[o+66]) via the shifted
                                # plane: planes 0 and 2, step 2
                                o = r * _WP + p
                                rr = (r - r0) * _W
                                nc.tensor.matmul(
                                    psum[:, rr:rr + _W],
                                    k_sb[fh][:, _NTAP + p],
                                    x_sb[:, 0:3:2, o:o + _W],
                                    start=False,
                                    stop=(p == _NPAIR - 1 and r == r1 - 1),
                                    perf_mode=DR,
                                )
                        # eviction: psum/64 + bias, alternating engines; the
                        # tiny first/last chunks and the 256-chunk get their
                        # own DMAs so the prologue starts and the tail drains
                        # fast
                        if ci == 0:
                            o_sb = osp.tile([_FH, n], f16,
                                            name=f"os0_s{s}f{fh}", tag="os0")
                            nc.scalar.activation(o_sb[:], psum[:], ident,
                                                 bias=bias_ap,
                                                 scale=1.0 / _KSCALE)
                            nc.sync.dma_start(
                                yT[s, fh, :, base:base + n], o_sb[:])
                        elif ci == nlast:
                            o_sb = osp.tile([_FH, n], f16,
                                            name=f"os1_s{s}f{fh}", tag="os1")
                            nc.vector.tensor_scalar(
                                o_sb[:], psum[:],
                                1.0 / _KSCALE, bias_ap, mult, add)
                            nc.sync.dma_start(
                                yT[s, fh, :, base:base + n], o_sb[:])
                        elif ci == nlast - 1:
                            o_sb = osp.tile([_FH, n], f16,
                                            name=f"os2_s{s}f{fh}", tag="os2")
                            nc.scalar.activation(o_sb[:], psum[:], ident,
                                                 bias=bias_ap,
                                                 scale=1.0 / _KSCALE)
                            nc.scalar.dma_start(
                                yT[s, fh, :, base:base + n], o_sb[:])
                        else:
                            bi = nbig
                            nbig += 1
                            on_dve = bi % 2 == 0
                            if bi == 6:        # unpaired 512-chunk
                                o_sb = osp.tile([_FH, 512], f16,
                                                name=f"ot_s{s}f{fh}",
                                                tag="ot")
                                dst = o_sb[:]
                            elif on_dve:       # first of a pair
                                o_pair = op.tile([_FH, 1024], f16,
                                                 name=f"o_s{s}f{fh}p{bi//2}",
                                                 tag="o")
                                dst = o_pair[:, 0:512]
                            else:              # second of a pair
                                dst = o_pair[:, 512:1024]
                            if on_dve:
                                nc.vector.tensor_scalar(
                                    dst, psum[:],
                                    1.0 / _KSCALE, bias_ap, mult, add)
                            else:
                                nc.scalar.activation(
                                    dst, psum[:], ident,
                                    bias=bias_ap, scale=1.0 / _KSCALE)
                            if bi == 6:
                                nc.sync.dma_start(
                                    yT[s, fh, :, base:base + 512], o_sb[:])
                            elif not on_dve:
                                nc.sync.dma_start(
                                    yT[s, fh, :, base - 512:base + 512],
                                    o_pair[:])
                gate_prev = gate_this
    nc.compile()
    return nc


def get_nc():
    global _nc
    if _nc is None:
        _nc = _build_nc()
    return _nc


def _prep_inputs(x, classes, kernel, bias):
    E = _e4m3()
    cls = np.asarray(classes)[:, 0]
    k_per = np.asarray(kernel)[cls]          # [B, KH, KW, CIN, F]
    b_per = np.asarray(bias)[cls]            # [B, F]

    # x -> padded flat planes, e4m3 hi/lo split + shifted-hi plane
    xpad = np.zeros((_B, _HP, _WP, _CIN), np.float32)
    xpad[:, 1:_H + 1, 1:_W + 1, :] = np.asarray(x, np.float32)
    xflat = np.ascontiguousarray(
        xpad.transpose(0, 3, 1, 2)).reshape(_B, _CIN, _FLAT)
    x_hi = xflat.astype(E)
    x_lo = (xflat - x_hi.astype(np.float32)).astype(E)
    x_sh = np.zeros_like(x_hi)
    x_sh[:, :, :_FLAT - _WP] = x_hi[:, :, _WP:]
    xT_all = np.stack([x_hi, x_lo, x_sh], axis=2)  # [B, CIN, 3, FLAT]

    # kernel -> 64x pre-scaled e4m3 hi/lo, packed into matmul slot tiles
    k64 = k_per.reshape(_B, _NTAP, _CIN, _NFH, _FH).astype(np.float32) * _KSCALE
    k_hi = k64.astype(E)
    k_lo = (k64 - k_hi.astype(np.float32)).astype(E)
    # [B, NFH, CIN, NTAP, FH]
    kA = np.ascontiguousarray(k_hi.transpose(0, 3, 2, 1, 4))
    kL = np.ascontiguousarray(k_lo.transpose(0, 3, 2, 1, 4))
    kT_all = np.zeros((_B, _NFH, _CIN, _NMM, 2, _FH), E)
    kT_all[:, :, :, :_NTAP, 0] = kA
    kT_all[:, :, :, :_NTAP, 1] = kA
    for p in range(_NPAIR):
        kT_all[:, :, :, _NTAP + p, 0] = kL[:, :, :, p]
        kT_all[:, :, :, _NTAP + p, 1] = kL[:, :, :, p + _KW]

    in_maps = []
    for i in range(_NCORES):
        lo = i * _BPC
        b_core = np.ascontiguousarray(
            b_per[lo:lo + _BPC].reshape(_BPC, _NFH, _FH)
            .astype(np.float32).transpose(2, 0, 1)
        ).reshape(_FH, _BPC * _NFH)
        in_maps.append({
            "xT": np.ascontiguousarray(xT_all[lo:lo + _BPC]),
            "kT": np.ascontiguousarray(kT_all[lo:lo + _BPC]),
            "bT": b_core,
        })
    return in_maps


def _unshard_one(yT):
    # yT: [BPC, NFH, FH, H*W] f16 -> [BPC, H, W, F] f32
    y = yT.astype(np.float32).reshape(_BPC, _F, _H * _W)
    return y.transpose(0, 2, 1).reshape(_BPC, _H, _W, _F)


def _unshard(results):
    outs = [_unshard_one(r["yT"]) for r in results]
    return np.ascontiguousarray(np.concatenate(outs, axis=0))


def run(x, classes, kernel, bias, trace=False):
    """Returns (y, BassKernelResults)."""
    from concourse.bass_utils import run_bass_kernel_spmd

    nc = get_nc()
    in_maps = _prep_inputs(x, classes, kernel, bias)
    res = run_bass_kernel_spmd(nc, in_maps, core_ids=list(range(_NCORES)),
                               trace=trace)
    return _unshard(res.results), res


def kernel(x, classes, kernel, bias):
    y, _ = run(x, classes, kernel, bias)
    return y
